# revision 39
# baseline (speedup 1.0000x reference)
"""MultiHeadTimeDimensionAttention kernel for Trainium2 (8 NeuronCores).

Math (per batch b):
  q[h,d]      = o_last[b] . Wq[h,:,d] + bq[h,d]
  scores[t,h] = sum_z o_all[b,t,z] * wkq[z,h]        (wkq[z,h] = sum_d Wk[h,z,d] q[h,d])
                (bk folds to a per-head constant -> softmax invariant -> dropped)
  p = exp(scores - max_t), l = sum_t p               (softmax unnormalized)
  r[h,z]      = sum_t p[t,h] * o_all[b,t,z]
  ctx[h,d]    = (sum_z r[h,z] Wv[h,z,d]) / l[h] + bv[h,d]

Exact algebraic restructure of the reference (einsum reassociation), ~64x
fewer FLOPs than materializing K/V. fp16 PE inputs (fp32 PSUM accumulation),
softmax max/exp in fp32 with fp16 shifted-score storage.

Sharding: data-parallel over B; each core handles B/8=2 batches.

Schedule notes (why it looks the way it does):
- All DMA rides one globally-serialized ring set (~358 GB/s), and each HWDGE
  DMA occupies its issuing engine for roughly the transfer time. XBAR DMA
  transposes additionally exclude all other DMA traffic and corrupt data if
  issued from both HWDGE engines concurrently -> A^T is produced entirely on
  the PE (fp16 128x128 transposes), NXB=0. (NXB>0 paths kept for reference.)
- sync carries olT/wq/wkT then the A block stream; scalar carries exps and
  half the PSUM->SBUF staging copies; vector the other half plus the fused
  per-block softmax drain (row max + shifted fp16 copy).
- Softmax is pipelined per 512-column block: per-block max/shift during the
  scores pass, then per-block exp -> p^T transposes -> r accumulation, so
  there is no monolithic max/exp stall. Batches overlap through buffer (WAR)
  rotation on the shared A blocks; blocks 0-1 are double-buffered so batch 1
  scores can start under batch 0's softmax.
- PSUM: 2 banks score accumulation, 3 banks A^T transpose staging, 1 bank
  p^T staging (shared with PE warmup), 2 banks r/ctx accumulation = 8.
- 60 dummy transposes at t=0 ramp the PE clock out of the low p-state while
  weights stream in. The ctx diagonal-block extraction ends with a contiguous
  halving fold in fp16 instead of a strided grouped reduce.
"""

import os
import numpy as np

import concourse.bacc as bacc
import concourse.tile as tile
import concourse.mybir as mybir
from concourse.bass_utils import run_bass_kernel_spmd
from concourse.masks import make_identity

B, T, Z, H, DK = 16, 4096, 1024, 16, 64
P = 128
NCORES = 8
BLOC = B // NCORES          # batches per core
ZC = Z // P                 # 8 z-chunks
NT = T // P                 # 32 t-tiles
TB = 512                    # t-block for scores pass
NTB = T // TB               # 8
NPAIR = H // 2              # 8 head-pairs
F32 = mybir.dt.float32
F16 = mybir.dt.float16
NXB = int(os.environ.get("NXB", "0"))   # z-chunks via XBAR DMA-transpose
XB_DEPTH = 4                            # XBAR prefetch depth (t-blocks)


def build_nc():
    nc = bacc.Bacc(None, target_bir_lowering=False)

    o16 = nc.declare_dram_parameter("o16", [BLOC, T, Z], F16, isOutput=False)
    ozb = (
        nc.declare_dram_parameter("ozb", [BLOC, NXB, T, P], F16, isOutput=False)
        if NXB > 0
        else None
    )
    o_lastT = nc.declare_dram_parameter("o_lastT", [P, ZC, BLOC], F16, isOutput=False)
    wq16 = nc.declare_dram_parameter("Wq16", [P, ZC, Z], F16, isOutput=False)
    wkT16 = nc.declare_dram_parameter("WkT16", [P, NPAIR, Z], F16, isOutput=False)
    wv16 = nc.declare_dram_parameter("Wv16", [P, ZC, Z], F16, isOutput=False)
    bq_r = nc.declare_dram_parameter("bq_r", [P, ZC], F32, isOutput=False)
    bv_in = nc.declare_dram_parameter("bv", [H, DK], F32, isOutput=False)
    dmask = nc.declare_dram_parameter("dmask", [H, Z], F32, isOutput=False)
    out = nc.declare_dram_parameter("out", [BLOC, Z], F32, isOutput=True)

    with tile.TileContext(nc) as tc:
        with (
            tc.tile_pool(name="const", bufs=1) as const,
            tc.tile_pool(name="small", bufs=1) as small,
            tc.tile_pool(name="abuf", bufs=1) as abuf,
            tc.tile_pool(name="stage", bufs=1) as stage,
            tc.tile_pool(name="xstage", bufs=XB_DEPTH) as xstage,
            tc.tile_pool(name="mpsum", bufs=2, space="PSUM") as mpsum,
            tc.tile_pool(name="tpsum", bufs=3, space="PSUM") as tpsum,
            tc.tile_pool(name="ppsum", bufs=1, space="PSUM") as ppsum,
            tc.tile_pool(name="rpsum", bufs=1, space="PSUM") as rpsum,
        ):
            ident16 = const.tile([P, P], F16)
            make_identity(nc, ident16)
            identh = const.tile([H, H], F16)
            make_identity(nc, identh)
            bv_sb = const.tile([H, DK], F32)
            nc.sync.dma_start(out=bv_sb, in_=bv_in[:])
            bqr_sb = const.tile([P, ZC], F32)
            nc.sync.dma_start(out=bqr_sb, in_=bq_r[:])
            dmask_sb = const.tile([H, Z], F32)
            nc.sync.dma_start(out=dmask_sb, in_=dmask[:])

            warm_ps = ppsum.tile([P, P], F16, tag="pp")
            for _ in range(60):
                nc.tensor.transpose(warm_ps, ident16, ident16)

            a_sb = {}  # (b, blk) -> tile; blocks 0-1 per-batch, 2-7 WAR-shared

            def load_a(b, blks):
                for blk in blks:
                    tag = f"a{b}_{blk}" if blk < 2 else f"a{blk}"
                    a_t = abuf.tile([P, 4, Z], F16, tag=tag, name=f"a{b}_{blk}")
                    nc.sync.dma_start(
                        out=a_t,
                        in_=o16[b, blk * TB : (blk + 1) * TB, :].rearrange(
                            "(i zp) z -> zp i z", zp=P
                        ),
                    )
                    a_sb[(b, blk)] = a_t

            # XBAR prefetch pump: one generation = NXB transposed tiles for
            # one (batch, t-block); stays XB_DEPTH generations ahead.
            xbar_plan = [(b, tb) for b in range(BLOC) for tb in range(NTB)]
            xb_tiles = {}
            xb_state = {"cursor": 0, "consumed": 0}

            def pump_xbar():
                if NXB == 0:
                    return
                while (
                    xb_state["cursor"] < len(xbar_plan)
                    and xb_state["cursor"] < xb_state["consumed"] + XB_DEPTH
                ):
                    bb, tt = xbar_plan[xb_state["cursor"]]
                    tiles = []
                    for x in range(NXB):
                        atx = xstage.tile([P, TB], F16, tag=f"x{x}", name=f"atx{x}")
                        nc.sync.dma_start_transpose(
                            atx, ozb[bb, x, tt * TB : (tt + 1) * TB, :]
                        )
                        tiles.append(atx)
                    xb_tiles[(bb, tt)] = tiles
                    xb_state["cursor"] += 1

            def take_xbar(b, tb):
                tiles = xb_tiles.pop((b, tb))
                xb_state["consumed"] += 1
                return tiles

            pe_zcs = list(range(NXB, ZC))
            pe_groups = [pe_zcs[i : i + 2] for i in range(0, len(pe_zcs), 2)]
            wkq_sb = []

            def scores_tb(b, tb, gis=None, sc_ps=None):
                """Accumulate scores^T[h, tb-block] for the given PE-transpose
                group indices (all by default); returns the psum tile."""
                full = gis is None
                if full:
                    gis = range(len(pe_groups))
                if sc_ps is None:
                    sc_ps = mpsum.tile([H, TB], F32, tag="sc")
                if full or 0 in gis:
                    xt = take_xbar(b, tb) if NXB > 0 else []
                    for x in range(NXB):
                        nc.tensor.matmul(
                            sc_ps,
                            wkq_sb[b][:, x, :],
                            xt[x],
                            start=(x == 0),
                            stop=(x == ZC - 1),
                        )
                for gi in gis:
                    grp = pe_groups[gi]
                    at_ps = tpsum.tile([P, 2 * TB], F16, tag="atps")
                    for j, zc in enumerate(grp):
                        for i in range(4):
                            nc.tensor.transpose(
                                at_ps[:, j * TB + i * P : j * TB + (i + 1) * P],
                                a_sb[(b, tb)][:, i, zc * P : (zc + 1) * P],
                                ident16,
                            )
                    at16 = stage.tile([P, 2 * TB], F16, tag="at16", bufs=4)
                    if gi % 2 == 0:
                        nc.vector.tensor_copy(
                            out=at16[:, : len(grp) * TB],
                            in_=at_ps[:, : len(grp) * TB],
                        )
                    else:
                        nc.scalar.copy(
                            out=at16[:, : len(grp) * TB],
                            in_=at_ps[:, : len(grp) * TB],
                        )
                    for j, zc in enumerate(grp):
                        nc.tensor.matmul(
                            sc_ps,
                            wkq_sb[b][:, zc, :],
                            at16[:, j * TB : (j + 1) * TB],
                            start=(NXB == 0 and zc == pe_zcs[0]),
                            stop=(zc == ZC - 1),
                        )
                return sc_ps

            pending_sc = {}

            # ------------- prologue: q and wkq for both batches --------------
            with tc.tile_pool(name="wpro", bufs=1) as wpro:
                olT_sb = wpro.tile([P, ZC, BLOC], F16)
                nc.sync.dma_start(out=olT_sb, in_=o_lastT[:])
                wq_sb = wpro.tile([P, ZC, Z], F16)
                for zc in range(ZC):
                    nc.sync.dma_start(out=wq_sb[:, zc, :], in_=wq16[:, zc, :])
                wkT_sb = wpro.tile([P, NPAIR, Z], F16)
                for pr in range(NPAIR):
                    nc.sync.dma_start(out=wkT_sb[:, pr, :], in_=wkT16[:, pr, :])
                load_a(0, [0, 1])
                pump_xbar()

                # q[m, b] (full vector H*DK=Z, chunked 128), fp32
                q_sb = wpro.tile([P, ZC, BLOC], F32)
                for mc in range(ZC):
                    qp = tpsum.tile([P, BLOC], F32, tag="atps")
                    for zc in range(ZC):
                        nc.tensor.matmul(
                            qp,
                            wq_sb[:, zc, mc * P : (mc + 1) * P],
                            olT_sb[:, zc, :],
                            start=(zc == 0),
                            stop=(zc == ZC - 1),
                        )
                    nc.vector.tensor_tensor(
                        q_sb[:, mc, :],
                        qp,
                        bqr_sb[:, mc : mc + 1].to_broadcast((P, BLOC)),
                        mybir.AluOpType.add,
                    )

                # head-split q, both batches: qsel[dd, pair, j, b]
                qsel = wpro.tile([P, NPAIR, 2, BLOC], F16)
                nc.vector.memset(qsel, 0.0)
                for b in range(BLOC):
                    for pr in range(NPAIR):
                        nc.vector.tensor_copy(
                            out=qsel[0:DK, pr, 0, b : b + 1],
                            in_=q_sb[0:DK, pr, b : b + 1],
                        )
                        nc.vector.tensor_copy(
                            out=qsel[DK:P, pr, 1, b : b + 1],
                            in_=q_sb[DK:P, pr, b : b + 1],
                        )

                for b in range(BLOC):
                    wkq_b = const.tile(
                        [P, ZC, H], F16, tag=f"wkq{b}", name=f"wkq{b}"
                    )
                    wkq_sb.append(wkq_b)
                def wkq_chunk(zcs):
                    for zc in zcs:
                        wp2 = tpsum.tile([P, NPAIR, 2, BLOC], F32, tag="atps")
                        for pr in range(NPAIR):
                            nc.tensor.matmul(
                                wp2[:, pr, :, :],
                                wkT_sb[:, pr, zc * P : (zc + 1) * P],
                                qsel[:, pr, :, :],
                                start=True,
                                stop=True,
                            )
                        for b in range(BLOC):
                            nc.vector.tensor_copy(
                                out=wkq_sb[b][:, zc, :].rearrange(
                                    "zp (pr j) -> zp pr j", pr=NPAIR
                                ),
                                in_=wp2[:, :, :, b],
                            )

                wkq_chunk(range(4))
                # first half of tb0's scores slots into the wkq shadow
                pending_sc[(0, 0)] = scores_tb(0, 0, gis=[0, 1])
                wkq_chunk(range(4, ZC))

            wv_sb = const.tile([P, ZC, Z], F16)

            # ------------- per-batch pipeline --------------------------------
            for b in range(BLOC):
                if b == 1:
                    load_a(b, [0, 1])

                # s16 holds scores shifted by the per-block max (values <= 0,
                # near-0 entries dominate the softmax -> fp16 is accurate),
                # then is overwritten in place by exp (= unnormalized p).
                s16 = stage.tile([H, T], F16, tag=f"s16_{b}", name=f"s16_{b}")
                m8 = small.tile([H, NTB], F32, tag=f"m8_{b}", name=f"m8_{b}")

                for tb in range(NTB):
                    pump_xbar()
                    if b == 0:
                        nxt = tb + 2
                        if 2 <= nxt < 8 and (b, nxt) not in a_sb:
                            load_a(b, [nxt])
                    elif tb < 3:
                        load_a(b, [2 * tb + 2, 2 * tb + 3])
                    if (b, tb) in pending_sc:
                        sc_ps = scores_tb(
                            b, tb, gis=[2, 3], sc_ps=pending_sc.pop((b, tb))
                        )
                    else:
                        sc_ps = scores_tb(b, tb)
                    nc.vector.reduce_max(
                        m8[:, tb : tb + 1], sc_ps, axis=mybir.AxisListType.X
                    )
                    nc.vector.tensor_scalar_sub(
                        out=s16[:, tb * TB : (tb + 1) * TB],
                        in0=sc_ps,
                        scalar1=m8[:, tb : tb + 1],
                    )

                mx = small.tile([H, 1], F32, tag=f"mx_{b}", name=f"mx_{b}")
                nc.vector.reduce_max(mx, m8, axis=mybir.AxisListType.X)
                # md8[:, tb] = m8[:, tb] - M  (bias for each exp block)
                md8 = small.tile([H, NTB], F32, tag=f"md_{b}", name=f"md_{b}")
                nc.vector.tensor_scalar_sub(out=md8, in0=m8, scalar1=mx)

                if b == 0:
                    # wv rides the rings after the startup burst, well before
                    # its first use at b0's ctx
                    for zc in range(ZC):
                        nc.sync.dma_start(out=wv_sb[:, zc, :], in_=wv16[:, zc, :])

                lsum8 = small.tile([H, NTB], F32, tag=f"l8_{b}", name=f"l8_{b}")
                p_sb = stage.tile(
                    [P, NT, H], F16, tag=f"psb_{b}", name=f"psb_{b}"
                )
                r_ps = rpsum.tile([H, 2, TB], F32, tag="racc")

                for tb in range(NTB):
                    pump_xbar()
                    # p = exp(s - m_tb + (m_tb - M)) in place, block by block
                    nc.scalar.activation(
                        out=s16[:, tb * TB : (tb + 1) * TB],
                        in_=s16[:, tb * TB : (tb + 1) * TB],
                        func=mybir.ActivationFunctionType.Exp,
                        bias=md8[:, tb : tb + 1],
                        scale=1.0,
                        accum_out=lsum8[:, tb : tb + 1],
                    )
                    pp = ppsum.tile([P, 4, H], F16, tag="pp")
                    for i in range(4):
                        tt = tb * 4 + i
                        nc.tensor.transpose(
                            pp[:, i, :], s16[:, tt * P : (tt + 1) * P], identh
                        )
                    nc.vector.tensor_copy(
                        out=p_sb[:, tb * 4 : (tb + 1) * 4, :], in_=pp
                    )
                    # r accumulation for this block's t-tiles
                    for i in range(4):
                        tt = tb * 4 + i
                        for zt in range(2):
                            nc.tensor.matmul(
                                r_ps[:, zt, :],
                                p_sb[:, tt, :],
                                a_sb[(b, tb)][:, i, zt * TB : (zt + 1) * TB],
                                start=(tt == 0),
                                stop=(tt == NT - 1),
                            )

                lsum = small.tile([H, 1], F32, tag=f"ls_{b}", name=f"ls_{b}")
                nc.vector.reduce_sum(lsum, lsum8, axis=mybir.AxisListType.X)
                rinv = small.tile([H, 1], F32, tag=f"ri_{b}", name=f"ri_{b}")
                nc.vector.reciprocal(rinv, lsum)

                r16 = small.tile([H, Z], F16, tag=f"r16_{b}", name=f"r16_{b}")
                nc.vector.tensor_copy(
                    out=r16, in_=r_ps.rearrange("h a f -> h (a f)")
                )
                rt_ps = ppsum.tile([P, ZC, H], F16, tag="pp")
                for zc in range(ZC):
                    nc.tensor.transpose(
                        rt_ps[:, zc, :], r16[:, zc * P : (zc + 1) * P], identh
                    )
                rt_sb = small.tile([P, ZC, H], F16, tag=f"rt_{b}", name=f"rt_{b}")
                nc.vector.tensor_copy(out=rt_sb, in_=rt_ps)

                # ctx_full[h', m] = sum_z r[h',z] WvF[z, m]; diag blocks kept
                cf_ps = rpsum.tile([H, 2, TB], F32, tag="racc")
                for mt in range(2):
                    for zc in range(ZC):
                        nc.tensor.matmul(
                            cf_ps[:, mt, :],
                            rt_sb[:, zc, :],
                            wv_sb[:, zc, mt * TB : (mt + 1) * TB],
                            start=(zc == 0),
                            stop=(zc == ZC - 1),
                        )
                # mask the off-diagonal head blocks (fp16 out), then reduce
                # the 16 blocks with a contiguous halving fold (cheaper than a
                # stride-64 grouped reduce_sum on DVE)
                masked = small.tile([H, Z], F16, tag="masked", bufs=2)
                nc.vector.tensor_tensor(
                    masked,
                    cf_ps.rearrange("h a f -> h (a f)"),
                    dmask_sb,
                    mybir.AluOpType.mult,
                )
                fold = small.tile([H, Z // 2], F16, tag="fold", bufs=2)
                w = Z // 2
                nc.vector.tensor_tensor(
                    fold[:, :w], masked[:, :w], masked[:, w:], mybir.AluOpType.add
                )
                while w > DK:
                    h2 = w // 2
                    nc.vector.tensor_tensor(
                        fold[:, :h2],
                        fold[:, :h2],
                        fold[:, h2:w],
                        mybir.AluOpType.add,
                    )
                    w = h2
                out_sb = small.tile([H, DK], F32, tag="outsb", bufs=2)
                nc.vector.tensor_scalar_mul(
                    out=out_sb, in0=fold[:, :DK], scalar1=rinv
                )
                nc.vector.tensor_add(out=out_sb, in0=out_sb, in1=bv_sb)
                nc.sync.dma_start(
                    out=out[b].rearrange("(h d) -> h d", h=H), in_=out_sb
                )

    nc.finalize()
    return nc


_NC_CACHE = {}


def _get_nc():
    if "nc" not in _NC_CACHE:
        _NC_CACHE["nc"] = build_nc()
    return _NC_CACHE["nc"]


def prep_inputs(o_all, o_last, Wk, Wv, Wq, bk, bv, bq):
    """Host-side shard + layout prep. Returns per-core input maps."""
    o_all = np.asarray(o_all, dtype=np.float32)
    o_last = np.asarray(o_last, dtype=np.float32)
    Wk = np.asarray(Wk, dtype=np.float32)
    Wv = np.asarray(Wv, dtype=np.float32)
    Wq = np.asarray(Wq, dtype=np.float32)
    bv = np.asarray(bv, dtype=np.float32)
    bq = np.asarray(bq, dtype=np.float32)

    wq_flat = Wq.transpose(1, 0, 2).reshape(Z, Z)
    wq16 = np.ascontiguousarray(
        wq_flat.reshape(ZC, P, Z).transpose(1, 0, 2)
    ).astype(np.float16)
    wkT16 = np.ascontiguousarray(
        Wk.transpose(0, 2, 1).reshape(NPAIR, P, Z).transpose(1, 0, 2)
    ).astype(np.float16)
    wv_flat = Wv.transpose(1, 0, 2).reshape(Z, Z)
    wv16 = np.ascontiguousarray(
        wv_flat.reshape(ZC, P, Z).transpose(1, 0, 2)
    ).astype(np.float16)
    bq_r = np.ascontiguousarray(bq.reshape(Z).reshape(ZC, P).T)  # [P, ZC]
    bv_c = np.ascontiguousarray(bv)
    dmask = np.zeros((H, Z), dtype=np.float32)
    for h in range(H):
        dmask[h, h * DK : (h + 1) * DK] = 1.0

    in_maps = []
    for c in range(NCORES):
        sl = slice(c * BLOC, (c + 1) * BLOC)
        o16c = o_all[sl].astype(np.float16)
        olT16 = np.ascontiguousarray(
            o_last[sl, 0, :].T.reshape(ZC, P, BLOC).transpose(1, 0, 2)
        ).astype(np.float16)
        m = {
                "o16": o16c,
                "o_lastT": olT16,
                "Wq16": wq16,
                "WkT16": wkT16,
                "Wv16": wv16,
                "bq_r": bq_r,
                "bv": bv_c,
                "dmask": dmask,
            }
        if NXB > 0:
            m["ozb"] = np.ascontiguousarray(
                o16c.reshape(BLOC, T, ZC, P).transpose(0, 2, 1, 3)[:, :NXB]
            )
        in_maps.append(m)
    return in_maps


def kernel(o_all, o_last, Wk, Wv, Wq, bk, bv, bq, _trace=False, _trace_kwargs=None):
    nc = _get_nc()
    in_maps = prep_inputs(o_all, o_last, Wk, Wv, Wq, bk, bv, bq)
    res = run_bass_kernel_spmd(
        nc, in_maps, core_ids=list(range(NCORES)), trace=_trace,
        **(_trace_kwargs or {}),
    )
    outs = [r["out"] for r in res.results]
    full = np.concatenate(outs, axis=0).reshape(B, 1, Z)
    if _trace:
        kernel.last_result = res
    return full


# revision 40
# speedup vs baseline: 1.0446x; 1.0446x over previous
"""MultiHeadTimeDimensionAttention kernel for Trainium2 (8 NeuronCores).

Math (per batch b):
  q[h,d]      = o_last[b] . Wq[h,:,d] + bq[h,d]
  scores[t,h] = sum_z o_all[b,t,z] * wkq[z,h]        (wkq[z,h] = sum_d Wk[h,z,d] q[h,d])
                (bk folds to a per-head constant -> softmax invariant -> dropped)
  p = exp(scores - max_t), l = sum_t p               (softmax unnormalized)
  r[h,z]      = sum_t p[t,h] * o_all[b,t,z]
  ctx[h,d]    = (sum_z r[h,z] Wv[h,z,d]) / l[h] + bv[h,d]

Exact algebraic restructure of the reference (einsum reassociation), ~64x
fewer FLOPs than materializing K/V. fp16 PE inputs (fp32 PSUM accumulation),
softmax max/exp in fp32 with fp16 shifted-score storage.

Sharding: data-parallel over B; each core handles B/8=2 batches.

Schedule notes (why it looks the way it does):
- All DMA rides one globally-serialized ring set (~358 GB/s), and each HWDGE
  DMA occupies its issuing engine for roughly the transfer time. XBAR DMA
  transposes additionally exclude all other DMA traffic and corrupt data if
  issued from both HWDGE engines concurrently -> A^T is produced entirely on
  the PE (fp16 128x128 transposes), NXB=0. (NXB>0 paths kept for reference.)
- sync carries olT/wq/wkT then the A block stream; scalar carries exps and
  half the PSUM->SBUF staging copies; vector the other half plus the fused
  per-block softmax drain (row max + shifted fp16 copy).
- Softmax is pipelined per 512-column block: per-block max/shift during the
  scores pass, then per-block exp -> p^T transposes -> r accumulation, so
  there is no monolithic max/exp stall. Batches overlap through buffer (WAR)
  rotation on the shared A blocks; blocks 0-1 are double-buffered so batch 1
  scores can start under batch 0's softmax.
- PSUM: 2 banks score accumulation, 3 banks A^T transpose staging, 1 bank
  p^T staging (shared with PE warmup), 2 banks r/ctx accumulation = 8.
- 60 dummy transposes at t=0 ramp the PE clock out of the low p-state while
  weights stream in. The ctx diagonal-block extraction ends with a contiguous
  halving fold in fp16 instead of a strided grouped reduce.
"""

import os
import numpy as np

import concourse.bacc as bacc
import concourse.tile as tile
import concourse.mybir as mybir
from concourse.bass_utils import run_bass_kernel_spmd
from concourse.masks import make_identity

B, T, Z, H, DK = 16, 4096, 1024, 16, 64
P = 128
NCORES = 8
BLOC = B // NCORES          # batches per core
ZC = Z // P                 # 8 z-chunks
NT = T // P                 # 32 t-tiles
TB = 512                    # t-block for scores pass
NTB = T // TB               # 8
NPAIR = H // 2              # 8 head-pairs
F32 = mybir.dt.float32
F16 = mybir.dt.float16
NXB = int(os.environ.get("NXB", "0"))   # z-chunks via XBAR DMA-transpose
XB_DEPTH = 4                            # XBAR prefetch depth (t-blocks)


def build_nc():
    nc = bacc.Bacc(None, target_bir_lowering=False)

    o16 = nc.declare_dram_parameter("o16", [BLOC, T, Z], F16, isOutput=False)
    ozb = (
        nc.declare_dram_parameter("ozb", [BLOC, NXB, T, P], F16, isOutput=False)
        if NXB > 0
        else None
    )
    o_lastT = nc.declare_dram_parameter("o_lastT", [P, ZC, BLOC], F16, isOutput=False)
    wq16 = nc.declare_dram_parameter("Wq16", [P, ZC, Z], F16, isOutput=False)
    wkT16 = nc.declare_dram_parameter("WkT16", [P, NPAIR, Z], F16, isOutput=False)
    wv16 = nc.declare_dram_parameter("Wv16", [P, ZC, Z], F16, isOutput=False)
    bq_r = nc.declare_dram_parameter("bq_r", [P, ZC], F32, isOutput=False)
    bv_in = nc.declare_dram_parameter("bv", [H, DK], F32, isOutput=False)
    dmask = nc.declare_dram_parameter("dmask", [H, Z], F32, isOutput=False)
    out = nc.declare_dram_parameter("out", [BLOC, Z], F32, isOutput=True)

    with tile.TileContext(nc) as tc:
        with (
            tc.tile_pool(name="const", bufs=1) as const,
            tc.tile_pool(name="small", bufs=1) as small,
            tc.tile_pool(name="abuf", bufs=1) as abuf,
            tc.tile_pool(name="stage", bufs=1) as stage,
            tc.tile_pool(name="xstage", bufs=XB_DEPTH) as xstage,
            tc.tile_pool(name="mpsum", bufs=2, space="PSUM") as mpsum,
            tc.tile_pool(name="tpsum", bufs=3, space="PSUM") as tpsum,
            tc.tile_pool(name="ppsum", bufs=1, space="PSUM") as ppsum,
            tc.tile_pool(name="rpsum", bufs=1, space="PSUM") as rpsum,
        ):
            ident16 = const.tile([P, P], F16)
            make_identity(nc, ident16)
            identh = const.tile([H, H], F16)
            make_identity(nc, identh)
            bv_sb = const.tile([H, DK], F32)
            nc.sync.dma_start(out=bv_sb, in_=bv_in[:])
            bqr_sb = const.tile([P, ZC], F32)
            nc.sync.dma_start(out=bqr_sb, in_=bq_r[:])
            dmask_sb = const.tile([H, Z], F32)
            nc.sync.dma_start(out=dmask_sb, in_=dmask[:])

            warm_ps = ppsum.tile([P, P], F16, tag="pp")
            for _ in range(60):
                nc.tensor.transpose(warm_ps, ident16, ident16)

            a_sb = {}  # (b, blk) -> tile; blocks 0-1 per-batch, 2-7 WAR-shared

            def load_a(b, blks):
                for blk in blks:
                    tag = f"a{b}_{blk}" if blk < 2 else f"a{blk}"
                    a_t = abuf.tile([P, 4, Z], F16, tag=tag, name=f"a{b}_{blk}")
                    nc.sync.dma_start(
                        out=a_t,
                        in_=o16[b, blk * TB : (blk + 1) * TB, :].rearrange(
                            "(i zp) z -> zp i z", zp=P
                        ),
                    )
                    a_sb[(b, blk)] = a_t

            # XBAR prefetch pump: one generation = NXB transposed tiles for
            # one (batch, t-block); stays XB_DEPTH generations ahead.
            xbar_plan = [(b, tb) for b in range(BLOC) for tb in range(NTB)]
            xb_tiles = {}
            xb_state = {"cursor": 0, "consumed": 0}

            def pump_xbar():
                if NXB == 0:
                    return
                while (
                    xb_state["cursor"] < len(xbar_plan)
                    and xb_state["cursor"] < xb_state["consumed"] + XB_DEPTH
                ):
                    bb, tt = xbar_plan[xb_state["cursor"]]
                    tiles = []
                    for x in range(NXB):
                        atx = xstage.tile([P, TB], F16, tag=f"x{x}", name=f"atx{x}")
                        nc.sync.dma_start_transpose(
                            atx, ozb[bb, x, tt * TB : (tt + 1) * TB, :]
                        )
                        tiles.append(atx)
                    xb_tiles[(bb, tt)] = tiles
                    xb_state["cursor"] += 1

            def take_xbar(b, tb):
                tiles = xb_tiles.pop((b, tb))
                xb_state["consumed"] += 1
                return tiles

            # ------------- prologue: q and wkq for both batches --------------
            wkq_sb = []
            with tc.tile_pool(name="wpro", bufs=1) as wpro:
                olT_sb = wpro.tile([P, ZC, BLOC], F16)
                nc.sync.dma_start(out=olT_sb, in_=o_lastT[:])
                wq_sb = wpro.tile([P, ZC, Z], F16)
                for zc in range(ZC):
                    nc.sync.dma_start(out=wq_sb[:, zc, :], in_=wq16[:, zc, :])
                wkT_sb = wpro.tile([P, NPAIR, Z], F16)
                for pr in range(NPAIR):
                    nc.sync.dma_start(out=wkT_sb[:, pr, :], in_=wkT16[:, pr, :])
                load_a(0, [0, 1])
                pump_xbar()

                # q[m, b] (full vector H*DK=Z, chunked 128), fp32
                q_sb = wpro.tile([P, ZC, BLOC], F32)
                for mc in range(ZC):
                    qp = tpsum.tile([P, BLOC], F32, tag="atps")
                    for zc in range(ZC):
                        nc.tensor.matmul(
                            qp,
                            wq_sb[:, zc, mc * P : (mc + 1) * P],
                            olT_sb[:, zc, :],
                            start=(zc == 0),
                            stop=(zc == ZC - 1),
                        )
                    nc.vector.tensor_tensor(
                        q_sb[:, mc, :],
                        qp,
                        bqr_sb[:, mc : mc + 1].to_broadcast((P, BLOC)),
                        mybir.AluOpType.add,
                    )

                # head-split q, both batches: qsel[dd, pair, j, b]
                qsel = wpro.tile([P, NPAIR, 2, BLOC], F16)
                nc.vector.memset(qsel, 0.0)
                for b in range(BLOC):
                    for pr in range(NPAIR):
                        nc.vector.tensor_copy(
                            out=qsel[0:DK, pr, 0, b : b + 1],
                            in_=q_sb[0:DK, pr, b : b + 1],
                        )
                        nc.vector.tensor_copy(
                            out=qsel[DK:P, pr, 1, b : b + 1],
                            in_=q_sb[DK:P, pr, b : b + 1],
                        )

                for b in range(BLOC):
                    wkq_b = const.tile(
                        [P, ZC, H], F16, tag=f"wkq{b}", name=f"wkq{b}"
                    )
                    wkq_sb.append(wkq_b)
                for zc in range(ZC):
                    wp2 = tpsum.tile([P, NPAIR, 2, BLOC], F32, tag="atps")
                    for pr in range(NPAIR):
                        nc.tensor.matmul(
                            wp2[:, pr, :, :],
                            wkT_sb[:, pr, zc * P : (zc + 1) * P],
                            qsel[:, pr, :, :],
                            start=True,
                            stop=True,
                        )
                    for b in range(BLOC):
                        nc.vector.tensor_copy(
                            out=wkq_sb[b][:, zc, :].rearrange(
                                "zp (pr j) -> zp pr j", pr=NPAIR
                            ),
                            in_=wp2[:, :, :, b],
                        )

            wv_sb = const.tile([P, ZC, Z], F16)

            # ------------- per-batch pipeline --------------------------------
            pe_zcs = list(range(NXB, ZC))
            pe_groups = [pe_zcs[i : i + 2] for i in range(0, len(pe_zcs), 2)]


            def scores_tb(b, tb):
                """Accumulate scores^T[h, tb-block]; returns psum tile."""
                xt = take_xbar(b, tb) if NXB > 0 else []
                sc_ps = mpsum.tile([H, TB], F32, tag="sc")
                for x in range(NXB):
                    nc.tensor.matmul(
                        sc_ps,
                        wkq_sb[b][:, x, :],
                        xt[x],
                        start=(x == 0),
                        stop=(x == ZC - 1),
                    )
                first = NXB == 0
                for gi, grp in enumerate(pe_groups):
                    at_ps = tpsum.tile([P, 2 * TB], F16, tag="atps")
                    for j, zc in enumerate(grp):
                        for i in range(4):
                            nc.tensor.transpose(
                                at_ps[:, j * TB + i * P : j * TB + (i + 1) * P],
                                a_sb[(b, tb)][:, i, zc * P : (zc + 1) * P],
                                ident16,
                            )
                    at16 = stage.tile([P, 2 * TB], F16, tag="at16", bufs=4)
                    if gi % 2 == 0:
                        nc.vector.tensor_copy(
                            out=at16[:, : len(grp) * TB],
                            in_=at_ps[:, : len(grp) * TB],
                        )
                    else:
                        nc.scalar.copy(
                            out=at16[:, : len(grp) * TB],
                            in_=at_ps[:, : len(grp) * TB],
                        )
                    for j, zc in enumerate(grp):
                        nc.tensor.matmul(
                            sc_ps,
                            wkq_sb[b][:, zc, :],
                            at16[:, j * TB : (j + 1) * TB],
                            start=first and zc == pe_zcs[0],
                            stop=(zc == ZC - 1),
                        )
                return sc_ps

            for b in range(BLOC):
                if b == 1:
                    load_a(b, [0, 1])

                # s16 holds scores shifted by the per-block max (values <= 0,
                # near-0 entries dominate the softmax -> fp16 is accurate),
                # then is overwritten in place by exp (= unnormalized p).
                s16 = stage.tile([H, T], F16, tag=f"s16_{b}", name=f"s16_{b}")
                m8 = small.tile([H, NTB], F32, tag=f"m8_{b}", name=f"m8_{b}")

                for tb in range(NTB):
                    pump_xbar()
                    if b == 0:
                        nxt = tb + 2
                        if 2 <= nxt < 8 and (b, nxt) not in a_sb:
                            load_a(b, [nxt])
                    elif tb < 3:
                        load_a(b, [2 * tb + 2, 2 * tb + 3])
                    sc_ps = scores_tb(b, tb)
                    nc.vector.reduce_max(
                        m8[:, tb : tb + 1], sc_ps, axis=mybir.AxisListType.X
                    )
                    nc.vector.tensor_scalar_sub(
                        out=s16[:, tb * TB : (tb + 1) * TB],
                        in0=sc_ps,
                        scalar1=m8[:, tb : tb + 1],
                    )

                mx = small.tile([H, 1], F32, tag=f"mx_{b}", name=f"mx_{b}")
                nc.vector.reduce_max(mx, m8, axis=mybir.AxisListType.X)
                # md8[:, tb] = m8[:, tb] - M  (bias for each exp block)
                md8 = small.tile([H, NTB], F32, tag=f"md_{b}", name=f"md_{b}")
                nc.vector.tensor_scalar_sub(out=md8, in0=m8, scalar1=mx)

                if b == 0:
                    # wv rides the rings after the startup burst, well before
                    # its first use at b0's ctx
                    for zc in range(ZC):
                        nc.sync.dma_start(out=wv_sb[:, zc, :], in_=wv16[:, zc, :])

                lsum8 = small.tile([H, NTB], F32, tag=f"l8_{b}", name=f"l8_{b}")
                p_sb = stage.tile(
                    [P, NT, H], F16, tag=f"psb_{b}", name=f"psb_{b}"
                )
                r_ps = rpsum.tile([H, 2, TB], F32, tag="racc")

                for tb in range(NTB):
                    pump_xbar()
                    # p = exp(s - m_tb + (m_tb - M)) in place, block by block
                    nc.scalar.activation(
                        out=s16[:, tb * TB : (tb + 1) * TB],
                        in_=s16[:, tb * TB : (tb + 1) * TB],
                        func=mybir.ActivationFunctionType.Exp,
                        bias=md8[:, tb : tb + 1],
                        scale=1.0,
                        accum_out=lsum8[:, tb : tb + 1],
                    )
                    pp = ppsum.tile([P, 4, H], F16, tag="pp")
                    for i in range(4):
                        tt = tb * 4 + i
                        nc.tensor.transpose(
                            pp[:, i, :], s16[:, tt * P : (tt + 1) * P], identh
                        )
                    nc.vector.tensor_copy(
                        out=p_sb[:, tb * 4 : (tb + 1) * 4, :], in_=pp
                    )
                    # r accumulation for this block's t-tiles
                    for i in range(4):
                        tt = tb * 4 + i
                        for zt in range(2):
                            nc.tensor.matmul(
                                r_ps[:, zt, :],
                                p_sb[:, tt, :],
                                a_sb[(b, tb)][:, i, zt * TB : (zt + 1) * TB],
                                start=(tt == 0),
                                stop=(tt == NT - 1),
                            )

                lsum = small.tile([H, 1], F32, tag=f"ls_{b}", name=f"ls_{b}")
                nc.vector.reduce_sum(lsum, lsum8, axis=mybir.AxisListType.X)
                rinv = small.tile([H, 1], F32, tag=f"ri_{b}", name=f"ri_{b}")
                nc.vector.reciprocal(rinv, lsum)

                r16 = small.tile([H, Z], F16, tag=f"r16_{b}", name=f"r16_{b}")
                nc.vector.tensor_copy(
                    out=r16, in_=r_ps.rearrange("h a f -> h (a f)")
                )
                rt_ps = ppsum.tile([P, ZC, H], F16, tag="pp")
                for zc in range(ZC):
                    nc.tensor.transpose(
                        rt_ps[:, zc, :], r16[:, zc * P : (zc + 1) * P], identh
                    )
                rt_sb = small.tile([P, ZC, H], F16, tag=f"rt_{b}", name=f"rt_{b}")
                nc.vector.tensor_copy(out=rt_sb, in_=rt_ps)

                # ctx_full[h', m] = sum_z r[h',z] WvF[z, m]; diag blocks kept
                cf_ps = rpsum.tile([H, 2, TB], F32, tag="racc")
                for mt in range(2):
                    for zc in range(ZC):
                        nc.tensor.matmul(
                            cf_ps[:, mt, :],
                            rt_sb[:, zc, :],
                            wv_sb[:, zc, mt * TB : (mt + 1) * TB],
                            start=(zc == 0),
                            stop=(zc == ZC - 1),
                        )
                # mask the off-diagonal head blocks (fp16 out), then reduce
                # the 16 blocks with a contiguous halving fold (cheaper than a
                # stride-64 grouped reduce_sum on DVE)
                masked = small.tile([H, Z], F16, tag="masked", bufs=2)
                nc.vector.tensor_tensor(
                    masked,
                    cf_ps.rearrange("h a f -> h (a f)"),
                    dmask_sb,
                    mybir.AluOpType.mult,
                )
                fold = small.tile([H, Z // 2], F16, tag="fold", bufs=2)
                w = Z // 2
                nc.vector.tensor_tensor(
                    fold[:, :w], masked[:, :w], masked[:, w:], mybir.AluOpType.add
                )
                while w > DK:
                    h2 = w // 2
                    nc.vector.tensor_tensor(
                        fold[:, :h2],
                        fold[:, :h2],
                        fold[:, h2:w],
                        mybir.AluOpType.add,
                    )
                    w = h2
                out_sb = small.tile([H, DK], F32, tag="outsb", bufs=2)
                nc.vector.tensor_scalar_mul(
                    out=out_sb, in0=fold[:, :DK], scalar1=rinv
                )
                nc.vector.tensor_add(out=out_sb, in0=out_sb, in1=bv_sb)
                nc.sync.dma_start(
                    out=out[b].rearrange("(h d) -> h d", h=H), in_=out_sb
                )

    nc.finalize()
    return nc


_NC_CACHE = {}


def _get_nc():
    if "nc" not in _NC_CACHE:
        _NC_CACHE["nc"] = build_nc()
    return _NC_CACHE["nc"]


def prep_inputs(o_all, o_last, Wk, Wv, Wq, bk, bv, bq):
    """Host-side shard + layout prep. Returns per-core input maps."""
    o_all = np.asarray(o_all, dtype=np.float32)
    o_last = np.asarray(o_last, dtype=np.float32)
    Wk = np.asarray(Wk, dtype=np.float32)
    Wv = np.asarray(Wv, dtype=np.float32)
    Wq = np.asarray(Wq, dtype=np.float32)
    bv = np.asarray(bv, dtype=np.float32)
    bq = np.asarray(bq, dtype=np.float32)

    wq_flat = Wq.transpose(1, 0, 2).reshape(Z, Z)
    wq16 = np.ascontiguousarray(
        wq_flat.reshape(ZC, P, Z).transpose(1, 0, 2)
    ).astype(np.float16)
    wkT16 = np.ascontiguousarray(
        Wk.transpose(0, 2, 1).reshape(NPAIR, P, Z).transpose(1, 0, 2)
    ).astype(np.float16)
    wv_flat = Wv.transpose(1, 0, 2).reshape(Z, Z)
    wv16 = np.ascontiguousarray(
        wv_flat.reshape(ZC, P, Z).transpose(1, 0, 2)
    ).astype(np.float16)
    bq_r = np.ascontiguousarray(bq.reshape(Z).reshape(ZC, P).T)  # [P, ZC]
    bv_c = np.ascontiguousarray(bv)
    dmask = np.zeros((H, Z), dtype=np.float32)
    for h in range(H):
        dmask[h, h * DK : (h + 1) * DK] = 1.0

    in_maps = []
    for c in range(NCORES):
        sl = slice(c * BLOC, (c + 1) * BLOC)
        o16c = o_all[sl].astype(np.float16)
        olT16 = np.ascontiguousarray(
            o_last[sl, 0, :].T.reshape(ZC, P, BLOC).transpose(1, 0, 2)
        ).astype(np.float16)
        m = {
                "o16": o16c,
                "o_lastT": olT16,
                "Wq16": wq16,
                "WkT16": wkT16,
                "Wv16": wv16,
                "bq_r": bq_r,
                "bv": bv_c,
                "dmask": dmask,
            }
        if NXB > 0:
            m["ozb"] = np.ascontiguousarray(
                o16c.reshape(BLOC, T, ZC, P).transpose(0, 2, 1, 3)[:, :NXB]
            )
        in_maps.append(m)
    return in_maps


def kernel(o_all, o_last, Wk, Wv, Wq, bk, bv, bq, _trace=False, _trace_kwargs=None):
    nc = _get_nc()
    in_maps = prep_inputs(o_all, o_last, Wk, Wv, Wq, bk, bv, bq)
    res = run_bass_kernel_spmd(
        nc, in_maps, core_ids=list(range(NCORES)), trace=_trace,
        **(_trace_kwargs or {}),
    )
    outs = [r["out"] for r in res.results]
    full = np.concatenate(outs, axis=0).reshape(B, 1, Z)
    if _trace:
        kernel.last_result = res
    return full


# revision 41
# speedup vs baseline: 1.0520x; 1.0070x over previous
"""MultiHeadTimeDimensionAttention kernel for Trainium2 (8 NeuronCores).

Math (per batch b):
  q[h,d]      = o_last[b] . Wq[h,:,d] + bq[h,d]
  scores[t,h] = sum_z o_all[b,t,z] * wkq[z,h]        (wkq[z,h] = sum_d Wk[h,z,d] q[h,d])
                (bk folds to a per-head constant -> softmax invariant -> dropped)
  p = exp(scores - max_t), l = sum_t p               (softmax unnormalized)
  r[h,z]      = sum_t p[t,h] * o_all[b,t,z]
  ctx[h,d]    = (sum_z r[h,z] Wv[h,z,d]) / l[h] + bv[h,d]

Exact algebraic restructure of the reference (einsum reassociation), ~64x
fewer FLOPs than materializing K/V. fp16 PE inputs (fp32 PSUM accumulation),
softmax max/exp in fp32 with fp16 shifted-score storage.

Sharding: data-parallel over B; each core handles B/8=2 batches.

Schedule notes (why it looks the way it does):
- All DMA rides one globally-serialized ring set (~358 GB/s), and each HWDGE
  DMA occupies its issuing engine for roughly the transfer time. XBAR DMA
  transposes additionally exclude all other DMA traffic and corrupt data if
  issued from both HWDGE engines concurrently -> A^T is produced entirely on
  the PE (fp16 128x128 transposes), NXB=0. (NXB>0 paths kept for reference.)
- sync carries olT/wq/wkT then the A block stream; scalar carries exps and
  half the PSUM->SBUF staging copies; vector the other half plus the fused
  per-block softmax drain (row max + shifted fp16 copy).
- Softmax is pipelined per 512-column block: per-block max/shift during the
  scores pass, then per-block exp -> p^T transposes -> r accumulation, so
  there is no monolithic max/exp stall. Batches overlap through buffer (WAR)
  rotation on the shared A blocks; blocks 0-1 are double-buffered so batch 1
  scores can start under batch 0's softmax.
- PSUM: 2 banks score accumulation, 3 banks A^T transpose staging, 1 bank
  p^T staging (shared with PE warmup), 2 banks r/ctx accumulation = 8.
- 60 dummy transposes at t=0 ramp the PE clock out of the low p-state while
  weights stream in. The ctx diagonal-block extraction ends with a contiguous
  halving fold in fp16 instead of a strided grouped reduce.
"""

import os
import numpy as np

import concourse.bacc as bacc
import concourse.tile as tile
import concourse.mybir as mybir
from concourse.bass_utils import run_bass_kernel_spmd
from concourse.masks import make_identity

B, T, Z, H, DK = 16, 4096, 1024, 16, 64
P = 128
NCORES = 8
BLOC = B // NCORES          # batches per core
ZC = Z // P                 # 8 z-chunks
NT = T // P                 # 32 t-tiles
TB = 512                    # t-block for scores pass
NTB = T // TB               # 8
NPAIR = H // 2              # 8 head-pairs
F32 = mybir.dt.float32
F16 = mybir.dt.float16
NXB = int(os.environ.get("NXB", "0"))   # z-chunks via XBAR DMA-transpose
XB_DEPTH = 4                            # XBAR prefetch depth (t-blocks)


def build_nc():
    nc = bacc.Bacc(None, target_bir_lowering=False)

    o16 = nc.declare_dram_parameter("o16", [BLOC, T, Z], F16, isOutput=False)
    ozb = (
        nc.declare_dram_parameter("ozb", [BLOC, NXB, T, P], F16, isOutput=False)
        if NXB > 0
        else None
    )
    o_lastT = nc.declare_dram_parameter("o_lastT", [P, ZC, BLOC], F16, isOutput=False)
    wq16 = nc.declare_dram_parameter("Wq16", [P, ZC, Z], F16, isOutput=False)
    wkT16 = nc.declare_dram_parameter("WkT16", [P, NPAIR, Z], F16, isOutput=False)
    wv16 = nc.declare_dram_parameter("Wv16", [P, ZC, Z], F16, isOutput=False)
    bq_r = nc.declare_dram_parameter("bq_r", [P, ZC], F32, isOutput=False)
    bv_in = nc.declare_dram_parameter("bv", [H, DK], F32, isOutput=False)
    dmask = nc.declare_dram_parameter("dmask", [H, Z], F32, isOutput=False)
    out = nc.declare_dram_parameter("out", [BLOC, Z], F32, isOutput=True)

    with tile.TileContext(nc) as tc:
        with (
            tc.tile_pool(name="const", bufs=1) as const,
            tc.tile_pool(name="small", bufs=1) as small,
            tc.tile_pool(name="abuf", bufs=1) as abuf,
            tc.tile_pool(name="stage", bufs=1) as stage,
            tc.tile_pool(name="xstage", bufs=XB_DEPTH) as xstage,
            tc.tile_pool(name="mpsum", bufs=2, space="PSUM") as mpsum,
            tc.tile_pool(name="tpsum", bufs=3, space="PSUM") as tpsum,
            tc.tile_pool(name="ppsum", bufs=1, space="PSUM") as ppsum,
            tc.tile_pool(name="rpsum", bufs=1, space="PSUM") as rpsum,
        ):
            ident16 = const.tile([P, P], F16)
            make_identity(nc, ident16)
            identh = const.tile([H, H], F16)
            make_identity(nc, identh)
            bv_sb = const.tile([H, DK], F32)
            nc.sync.dma_start(out=bv_sb, in_=bv_in[:])
            bqr_sb = const.tile([P, ZC], F32)
            nc.sync.dma_start(out=bqr_sb, in_=bq_r[:])
            dmask_sb = const.tile([H, Z], F32)
            nc.sync.dma_start(out=dmask_sb, in_=dmask[:])

            warm_ps = ppsum.tile([P, P], F16, tag="pp")
            for _ in range(60):
                nc.tensor.transpose(warm_ps, ident16, ident16)

            a_sb = {}  # (b, blk) -> tile; blocks 0-1 per-batch, 2-7 WAR-shared

            def load_a(b, blks):
                for blk in blks:
                    tag = f"a{b}_{blk}" if blk < 2 else f"a{blk}"
                    a_t = abuf.tile([P, 4, Z], F16, tag=tag, name=f"a{b}_{blk}")
                    nc.sync.dma_start(
                        out=a_t,
                        in_=o16[b, blk * TB : (blk + 1) * TB, :].rearrange(
                            "(i zp) z -> zp i z", zp=P
                        ),
                    )
                    a_sb[(b, blk)] = a_t

            # XBAR prefetch pump: one generation = NXB transposed tiles for
            # one (batch, t-block); stays XB_DEPTH generations ahead.
            xbar_plan = [(b, tb) for b in range(BLOC) for tb in range(NTB)]
            xb_tiles = {}
            xb_state = {"cursor": 0, "consumed": 0}

            def pump_xbar():
                if NXB == 0:
                    return
                while (
                    xb_state["cursor"] < len(xbar_plan)
                    and xb_state["cursor"] < xb_state["consumed"] + XB_DEPTH
                ):
                    bb, tt = xbar_plan[xb_state["cursor"]]
                    tiles = []
                    for x in range(NXB):
                        atx = xstage.tile([P, TB], F16, tag=f"x{x}", name=f"atx{x}")
                        nc.sync.dma_start_transpose(
                            atx, ozb[bb, x, tt * TB : (tt + 1) * TB, :]
                        )
                        tiles.append(atx)
                    xb_tiles[(bb, tt)] = tiles
                    xb_state["cursor"] += 1

            def take_xbar(b, tb):
                tiles = xb_tiles.pop((b, tb))
                xb_state["consumed"] += 1
                return tiles

            # ------------- prologue: q and wkq for both batches --------------
            wkq_sb = []
            with tc.tile_pool(name="wpro", bufs=1) as wpro:
                olT_sb = wpro.tile([P, ZC, BLOC], F16)
                nc.sync.dma_start(out=olT_sb, in_=o_lastT[:])
                wq_sb = wpro.tile([P, ZC, Z], F16)
                for zc in range(ZC):
                    nc.sync.dma_start(out=wq_sb[:, zc, :], in_=wq16[:, zc, :])
                wkT_sb = wpro.tile([P, NPAIR, Z], F16)
                for pr in range(NPAIR):
                    nc.sync.dma_start(out=wkT_sb[:, pr, :], in_=wkT16[:, pr, :])
                load_a(0, [0, 1])
                pump_xbar()

                # q[m, b] (full vector H*DK=Z, chunked 128), fp32
                q_sb = wpro.tile([P, ZC, BLOC], F32)
                for mc in range(ZC):
                    qp = tpsum.tile([P, BLOC], F32, tag="atps")
                    for zc in range(ZC):
                        nc.tensor.matmul(
                            qp,
                            wq_sb[:, zc, mc * P : (mc + 1) * P],
                            olT_sb[:, zc, :],
                            start=(zc == 0),
                            stop=(zc == ZC - 1),
                        )
                    nc.vector.tensor_tensor(
                        q_sb[:, mc, :],
                        qp,
                        bqr_sb[:, mc : mc + 1].to_broadcast((P, BLOC)),
                        mybir.AluOpType.add,
                    )

                # head-split q, both batches: qsel[dd, pair, j, b]
                qsel = wpro.tile([P, NPAIR, 2, BLOC], F16)
                nc.vector.memset(qsel, 0.0)
                for b in range(BLOC):
                    for pr in range(NPAIR):
                        nc.vector.tensor_copy(
                            out=qsel[0:DK, pr, 0, b : b + 1],
                            in_=q_sb[0:DK, pr, b : b + 1],
                        )
                        nc.vector.tensor_copy(
                            out=qsel[DK:P, pr, 1, b : b + 1],
                            in_=q_sb[DK:P, pr, b : b + 1],
                        )

                for b in range(BLOC):
                    wkq_b = const.tile(
                        [P, ZC, H], F16, tag=f"wkq{b}", name=f"wkq{b}"
                    )
                    wkq_sb.append(wkq_b)
                for zc in range(ZC):
                    wp2 = tpsum.tile([P, NPAIR, 2, BLOC], F32, tag="atps")
                    for pr in range(NPAIR):
                        nc.tensor.matmul(
                            wp2[:, pr, :, :],
                            wkT_sb[:, pr, zc * P : (zc + 1) * P],
                            qsel[:, pr, :, :],
                            start=True,
                            stop=True,
                        )
                    for b in range(BLOC):
                        nc.vector.tensor_copy(
                            out=wkq_sb[b][:, zc, :].rearrange(
                                "zp (pr j) -> zp pr j", pr=NPAIR
                            ),
                            in_=wp2[:, :, :, b],
                        )

            wv_sb = const.tile([P, ZC, Z], F16)

            # ------------- per-batch pipeline --------------------------------
            pe_zcs = list(range(NXB, ZC))
            pe_groups = [pe_zcs[i : i + 2] for i in range(0, len(pe_zcs), 2)]


            def scores_tb(b, tb):
                """Accumulate scores^T[h, tb-block]; returns psum tile."""
                xt = take_xbar(b, tb) if NXB > 0 else []
                sc_ps = mpsum.tile([H, TB], F32, tag="sc")
                for x in range(NXB):
                    nc.tensor.matmul(
                        sc_ps,
                        wkq_sb[b][:, x, :],
                        xt[x],
                        start=(x == 0),
                        stop=(x == ZC - 1),
                    )
                first = NXB == 0
                for gi, grp in enumerate(pe_groups):
                    at_ps = tpsum.tile([P, 2 * TB], F16, tag="atps")
                    for j, zc in enumerate(grp):
                        for i in range(4):
                            nc.tensor.transpose(
                                at_ps[:, j * TB + i * P : j * TB + (i + 1) * P],
                                a_sb[(b, tb)][:, i, zc * P : (zc + 1) * P],
                                ident16,
                            )
                    at16 = stage.tile([P, 2 * TB], F16, tag="at16", bufs=4)
                    if gi % 2 == 0:
                        nc.vector.tensor_copy(
                            out=at16[:, : len(grp) * TB],
                            in_=at_ps[:, : len(grp) * TB],
                        )
                    else:
                        nc.scalar.copy(
                            out=at16[:, : len(grp) * TB],
                            in_=at_ps[:, : len(grp) * TB],
                        )
                    for j, zc in enumerate(grp):
                        nc.tensor.matmul(
                            sc_ps,
                            wkq_sb[b][:, zc, :],
                            at16[:, j * TB : (j + 1) * TB],
                            start=first and zc == pe_zcs[0],
                            stop=(zc == ZC - 1),
                        )
                return sc_ps

            for b in range(BLOC):
                # s16 holds scores shifted by the per-block max (values <= 0,
                # near-0 entries dominate the softmax -> fp16 is accurate),
                # then is overwritten in place by exp (= unnormalized p).
                s16 = stage.tile([H, T], F16, tag=f"s16_{b}", name=f"s16_{b}")
                m8 = small.tile([H, NTB], F32, tag=f"m8_{b}", name=f"m8_{b}")

                for tb in range(NTB):
                    pump_xbar()
                    if b == 0:
                        nxt = tb + 2
                        if 2 <= nxt < 8 and (b, nxt) not in a_sb:
                            load_a(b, [nxt])
                        if tb in (6, 7):
                            # b1's dedicated blocks prefetch with no WAR hazard
                            load_a(1, [tb - 6])
                    elif tb < 3:
                        load_a(b, [2 * tb + 2, 2 * tb + 3])
                    sc_ps = scores_tb(b, tb)
                    nc.vector.reduce_max(
                        m8[:, tb : tb + 1], sc_ps, axis=mybir.AxisListType.X
                    )
                    nc.vector.tensor_scalar_sub(
                        out=s16[:, tb * TB : (tb + 1) * TB],
                        in0=sc_ps,
                        scalar1=m8[:, tb : tb + 1],
                    )

                mx = small.tile([H, 1], F32, tag=f"mx_{b}", name=f"mx_{b}")
                nc.vector.reduce_max(mx, m8, axis=mybir.AxisListType.X)
                # md8[:, tb] = m8[:, tb] - M  (bias for each exp block)
                md8 = small.tile([H, NTB], F32, tag=f"md_{b}", name=f"md_{b}")
                nc.vector.tensor_scalar_sub(out=md8, in0=m8, scalar1=mx)

                if b == 0:
                    # wv rides the rings after the startup burst, well before
                    # its first use at b0's ctx
                    for zc in range(ZC):
                        nc.sync.dma_start(out=wv_sb[:, zc, :], in_=wv16[:, zc, :])

                lsum8 = small.tile([H, NTB], F32, tag=f"l8_{b}", name=f"l8_{b}")
                p_sb = stage.tile(
                    [P, NT, H], F16, tag=f"psb_{b}", name=f"psb_{b}"
                )
                r_ps = rpsum.tile([H, 2, TB], F32, tag="racc")

                for tb in range(NTB):
                    pump_xbar()
                    # p = exp(s - m_tb + (m_tb - M)) in place, block by block
                    nc.scalar.activation(
                        out=s16[:, tb * TB : (tb + 1) * TB],
                        in_=s16[:, tb * TB : (tb + 1) * TB],
                        func=mybir.ActivationFunctionType.Exp,
                        bias=md8[:, tb : tb + 1],
                        scale=1.0,
                        accum_out=lsum8[:, tb : tb + 1],
                    )
                    pp = ppsum.tile([P, 4, H], F16, tag="pp")
                    for i in range(4):
                        tt = tb * 4 + i
                        nc.tensor.transpose(
                            pp[:, i, :], s16[:, tt * P : (tt + 1) * P], identh
                        )
                    nc.vector.tensor_copy(
                        out=p_sb[:, tb * 4 : (tb + 1) * 4, :], in_=pp
                    )
                    # r accumulation for this block's t-tiles
                    for i in range(4):
                        tt = tb * 4 + i
                        for zt in range(2):
                            nc.tensor.matmul(
                                r_ps[:, zt, :],
                                p_sb[:, tt, :],
                                a_sb[(b, tb)][:, i, zt * TB : (zt + 1) * TB],
                                start=(tt == 0),
                                stop=(tt == NT - 1),
                            )

                lsum = small.tile([H, 1], F32, tag=f"ls_{b}", name=f"ls_{b}")
                nc.vector.reduce_sum(lsum, lsum8, axis=mybir.AxisListType.X)
                rinv = small.tile([H, 1], F32, tag=f"ri_{b}", name=f"ri_{b}")
                nc.vector.reciprocal(rinv, lsum)

                r16 = small.tile([H, Z], F16, tag=f"r16_{b}", name=f"r16_{b}")
                nc.vector.tensor_copy(
                    out=r16, in_=r_ps.rearrange("h a f -> h (a f)")
                )
                rt_ps = ppsum.tile([P, ZC, H], F16, tag="pp")
                for zc in range(ZC):
                    nc.tensor.transpose(
                        rt_ps[:, zc, :], r16[:, zc * P : (zc + 1) * P], identh
                    )
                rt_sb = small.tile([P, ZC, H], F16, tag=f"rt_{b}", name=f"rt_{b}")
                nc.vector.tensor_copy(out=rt_sb, in_=rt_ps)

                # ctx_full[h', m] = sum_z r[h',z] WvF[z, m]; diag blocks kept
                cf_ps = rpsum.tile([H, 2, TB], F32, tag="racc")
                for mt in range(2):
                    for zc in range(ZC):
                        nc.tensor.matmul(
                            cf_ps[:, mt, :],
                            rt_sb[:, zc, :],
                            wv_sb[:, zc, mt * TB : (mt + 1) * TB],
                            start=(zc == 0),
                            stop=(zc == ZC - 1),
                        )
                # mask the off-diagonal head blocks (fp16 out), then reduce
                # the 16 blocks with a contiguous halving fold (cheaper than a
                # stride-64 grouped reduce_sum on DVE)
                masked = small.tile([H, Z], F16, tag="masked", bufs=2)
                nc.vector.tensor_tensor(
                    masked,
                    cf_ps.rearrange("h a f -> h (a f)"),
                    dmask_sb,
                    mybir.AluOpType.mult,
                )
                fold = small.tile([H, Z // 2], F16, tag="fold", bufs=2)
                w = Z // 2
                nc.vector.tensor_tensor(
                    fold[:, :w], masked[:, :w], masked[:, w:], mybir.AluOpType.add
                )
                while w > DK:
                    h2 = w // 2
                    nc.vector.tensor_tensor(
                        fold[:, :h2],
                        fold[:, :h2],
                        fold[:, h2:w],
                        mybir.AluOpType.add,
                    )
                    w = h2
                out_sb = small.tile([H, DK], F32, tag="outsb", bufs=2)
                nc.vector.tensor_scalar_mul(
                    out=out_sb, in0=fold[:, :DK], scalar1=rinv
                )
                nc.vector.tensor_add(out=out_sb, in0=out_sb, in1=bv_sb)
                nc.sync.dma_start(
                    out=out[b].rearrange("(h d) -> h d", h=H), in_=out_sb
                )

    nc.finalize()
    return nc


_NC_CACHE = {}


def _get_nc():
    if "nc" not in _NC_CACHE:
        _NC_CACHE["nc"] = build_nc()
    return _NC_CACHE["nc"]


def prep_inputs(o_all, o_last, Wk, Wv, Wq, bk, bv, bq):
    """Host-side shard + layout prep. Returns per-core input maps."""
    o_all = np.asarray(o_all, dtype=np.float32)
    o_last = np.asarray(o_last, dtype=np.float32)
    Wk = np.asarray(Wk, dtype=np.float32)
    Wv = np.asarray(Wv, dtype=np.float32)
    Wq = np.asarray(Wq, dtype=np.float32)
    bv = np.asarray(bv, dtype=np.float32)
    bq = np.asarray(bq, dtype=np.float32)

    wq_flat = Wq.transpose(1, 0, 2).reshape(Z, Z)
    wq16 = np.ascontiguousarray(
        wq_flat.reshape(ZC, P, Z).transpose(1, 0, 2)
    ).astype(np.float16)
    wkT16 = np.ascontiguousarray(
        Wk.transpose(0, 2, 1).reshape(NPAIR, P, Z).transpose(1, 0, 2)
    ).astype(np.float16)
    wv_flat = Wv.transpose(1, 0, 2).reshape(Z, Z)
    wv16 = np.ascontiguousarray(
        wv_flat.reshape(ZC, P, Z).transpose(1, 0, 2)
    ).astype(np.float16)
    bq_r = np.ascontiguousarray(bq.reshape(Z).reshape(ZC, P).T)  # [P, ZC]
    bv_c = np.ascontiguousarray(bv)
    dmask = np.zeros((H, Z), dtype=np.float32)
    for h in range(H):
        dmask[h, h * DK : (h + 1) * DK] = 1.0

    in_maps = []
    for c in range(NCORES):
        sl = slice(c * BLOC, (c + 1) * BLOC)
        o16c = o_all[sl].astype(np.float16)
        olT16 = np.ascontiguousarray(
            o_last[sl, 0, :].T.reshape(ZC, P, BLOC).transpose(1, 0, 2)
        ).astype(np.float16)
        m = {
                "o16": o16c,
                "o_lastT": olT16,
                "Wq16": wq16,
                "WkT16": wkT16,
                "Wv16": wv16,
                "bq_r": bq_r,
                "bv": bv_c,
                "dmask": dmask,
            }
        if NXB > 0:
            m["ozb"] = np.ascontiguousarray(
                o16c.reshape(BLOC, T, ZC, P).transpose(0, 2, 1, 3)[:, :NXB]
            )
        in_maps.append(m)
    return in_maps


def kernel(o_all, o_last, Wk, Wv, Wq, bk, bv, bq, _trace=False, _trace_kwargs=None):
    nc = _get_nc()
    in_maps = prep_inputs(o_all, o_last, Wk, Wv, Wq, bk, bv, bq)
    res = run_bass_kernel_spmd(
        nc, in_maps, core_ids=list(range(NCORES)), trace=_trace,
        **(_trace_kwargs or {}),
    )
    outs = [r["out"] for r in res.results]
    full = np.concatenate(outs, axis=0).reshape(B, 1, Z)
    if _trace:
        kernel.last_result = res
    return full
